# revision 12
# baseline (speedup 1.0000x reference)
"""CrossViewTransformer Trainium2 kernel (v2).

Shards batch B=4 x row-halves over 8 NeuronCores. Per core:
  q = Wq @ cross_ext   (32, 2176)  fp16 hi/lo split MMs (exact-ish)
  k = Wk @ front_x     (32, 4096)  same
  energy[j,i] = <q_j,k_i> via K=128 fp16 [qh;ql;qh;ql]x[kh;kh;kl;kl] MMs
  argmax: Act copies psum->SBUF, DVE reduce-max + is_equal mask + stt accum
  v computed key-major -> DRAM fp16; T gathered via dma_gather(transpose)
  conv3x3([front; T]) * S + front, ob=0 front-half overlapped with energy

All weights host-pre-transposed to fp16; activations host-split hi/lo fp16;
front cat rows host-padded; iota/masks host-built.
"""
import os
import sys

sys.path.insert(0, "/opt/trn_rl_repo")
import numpy as np  # noqa: E402
import concourse.bacc as bacc  # noqa: E402
import concourse.mybir as mybir  # noqa: E402
import concourse.tile as tile  # noqa: E402
from concourse import bass_utils  # noqa: E402

dt = mybir.dt
ALU = mybir.AluOpType
AX = mybir.AxisListType

B, C, H, W = 4, 256, 64, 64
C8 = C // 8            # 32
HWF = H * W            # 4096 keys
RH = H // 2            # 32 out rows per core
EXTR = RH + 2          # 34 ext rows
EXTQ = EXTR * W        # 2176 ext queries
NBLK = EXTQ // 128     # 17 query blocks
OUTP = RH * W          # 2048 out positions
WP = W + 2             # 66 padded width
VROWS = HWF + 1        # vdram rows (last = zero row)

_CACHED = {}


def _build(has_bqk: bool, has_bv: bool):
    key = (has_bqk, has_bv)
    if key in _CACHED:
        return _CACHED[key]
    nc = bacc.Bacc("TRN2", debug=False)

    cxh_d = nc.dram_tensor("cxh", (2, 128, EXTQ), dt.float16, kind="ExternalInput")
    cxl_d = nc.dram_tensor("cxl", (2, 128, EXTQ), dt.float16, kind="ExternalInput")
    fxh_d = nc.dram_tensor("fxh", (2, 128, HWF), dt.float16, kind="ExternalInput")
    fxl_d = nc.dram_tensor("fxl", (2, 128, HWF), dt.float16, kind="ExternalInput")
    xh_d = nc.dram_tensor("xh", (2, 128, HWF), dt.float16, kind="ExternalInput")
    fpad_d = nc.dram_tensor("fpad", (2, 128, EXTR, WP), dt.float16, kind="ExternalInput")
    wqkT_d = nc.dram_tensor("wqkT", (128, 8 * C8), dt.float16, kind="ExternalInput")
    wvm_d = nc.dram_tensor("wvm", (2, 128, C), dt.float16, kind="ExternalInput")
    wfT_d = nc.dram_tensor("wfT", (128, 72 * 128), dt.float16, kind="ExternalInput")
    iota_d = nc.dram_tensor("iota", (128, HWF), dt.int16, kind="ExternalInput")
    qb8_d = nc.dram_tensor("qb8", (128, NBLK), dt.float32, kind="ExternalInput")
    mask_d = nc.dram_tensor("mask", (128, NBLK), dt.float32, kind="ExternalInput")
    amask_d = nc.dram_tensor("amask", (128, NBLK), dt.float32, kind="ExternalInput")
    bq_d = nc.dram_tensor("bq", (C8, 1), dt.float32, kind="ExternalInput")
    bk_d = nc.dram_tensor("bk", (C8, 1), dt.float32, kind="ExternalInput")
    bv_d = nc.dram_tensor("bv", (128, 2), dt.float32, kind="ExternalInput")
    bf_d = nc.dram_tensor("bf", (128, 2), dt.float32, kind="ExternalInput")
    id_d = nc.dram_tensor("ident", (128, 128), dt.float32, kind="ExternalInput")

    out_d = nc.dram_tensor("out", (2, 128, OUTP), dt.float32, kind="ExternalOutput")
    dbg_arg_d = nc.dram_tensor("dbg_arg", (128, NBLK), dt.float32, kind="ExternalOutput")
    dbg_s_d = nc.dram_tensor("dbg_s", (128, NBLK), dt.float32, kind="ExternalOutput")

    with tile.TileContext(nc) as tc:
        _body(nc, tc, locals(), has_bqk, has_bv)
    nc.compile()
    _CACHED[key] = nc
    return nc


def _body(nc, tc, T, has_bqk, has_bv):
    F32, F16, I16 = dt.float32, dt.float16, dt.int16

    with tc.tile_pool(name="dramscr", bufs=1, space="DRAM") as DR, \
         tc.tile_pool(name="persist", bufs=1) as P, \
         tc.tile_pool(name="stream", bufs=2) as S:

        # ---------- persistent tiles ----------
        wqkT = P.tile([128, 8 * C8], F16, tag="wqkT")
        wvm = P.tile([128, 2, C], F16, tag="wvm")
        wfT = P.tile([128, 72 * 128], F16, tag="wfT")
        iot = P.tile([128, HWF], I16, tag="iota")
        ident = P.tile([128, 128], F32, tag="ident")
        qstack = P.tile([128, EXTQ], F16, tag="qstack")
        kstack = P.tile([128, HWF], F16, tag="kstack")
        fr0 = P.tile([128, EXTR, WP], F16, tag="fr0")
        fr1 = P.tile([128, EXTR, WP], F16, tag="fr1")
        ct2 = P.tile([128, EXTR, WP], F16, tag="ct2")
        ct3 = P.tile([128, EXTR, WP], F16, tag="ct3")
        s128 = P.tile([128, EXTQ], F32, tag="s128")
        tgp = [P.tile([128, 2, min(512, EXTQ - q0)], F16, tag=f"tg{i}",
                      name=f"tg{i}")
               for i, q0 in enumerate(range(0, EXTQ, 512))]
        idxw = P.tile([128, EXTQ // 16], I16, tag="idxw")
        widxP = P.tile([128, EXTQ // 16], I16, tag="widxP")
        qb8 = P.tile([128, NBLK], F32, tag="qb8")
        SM = P.tile([128, 128], F32, tag="smalls")
        vdram = DR.tile([VROWS, C], F16, tag="vdram")
        edram = DR.tile([NBLK * 1024, 512], F32, tag="edram")
        wrap_t = DR.tile([EXTQ], I16, tag="wrapl")
        wrapW = DR.tile([EXTQ], I16, tag="wrapW")
        srow_t = DR.tile([EXTQ], F32, tag="srowd")

        Mg = SM[:, 0:NBLK]
        Agf = SM[:, 17:17 + NBLK]
        Agm = SM[:, 34:34 + NBLK]
        maskt = SM[:, 51:51 + NBLK]
        amaskt = SM[:, 68:68 + NBLK]
        bqs = SM[0:C8, 85:87]
        bvs = SM[:, 87:89]
        bfs = SM[:, 89:91]

        # ---------- input DMAs (qk-critical first) ----------
        nc.sync.dma_start(wqkT[:, :], T["wqkT_d"].ap())
        nc.sync.dma_start(ident[:, :], T["id_d"].ap())
        nc.sync.dma_start(bqs[:, 0:1], T["bq_d"].ap())
        nc.sync.dma_start(bqs[:, 1:2], T["bk_d"].ap())
        nc.sync.dma_start(bvs[:, :], T["bv_d"].ap())
        nc.sync.dma_start(bfs[:, :], T["bf_d"].ap())
        nc.sync.dma_start(maskt[:, :], T["mask_d"].ap())
        nc.sync.dma_start(amaskt[:, :], T["amask_d"].ap())

        QKP_cm = tc.tile_pool(name="qkin", bufs=1)
        QKP = QKP_cm.__enter__()
        cxh = QKP.tile([128, 2, EXTQ], F16, tag="cxh")
        cxl = QKP.tile([128, 2, EXTQ], F16, tag="cxl")
        fxh = QKP.tile([128, 2, HWF], F16, tag="fxh")
        fxl = QKP.tile([128, 2, HWF], F16, tag="fxl")
        nc.sync.dma_start(cxh[:, :, :], T["cxh_d"].ap().rearrange("c p q -> p c q"))
        nc.sync.dma_start(cxl[:, :, :], T["cxl_d"].ap().rearrange("c p q -> p c q"))
        nc.sync.dma_start(fxh[:, :, :], T["fxh_d"].ap().rearrange("c p q -> p c q"))
        nc.sync.dma_start(fxl[:, :, :], T["fxl_d"].ap().rearrange("c p q -> p c q"))

        XHP_cm = tc.tile_pool(name="xhin", bufs=1)
        XHP = XHP_cm.__enter__()
        xh16 = XHP.tile([128, 2, HWF], F16, tag="xh16")
        nc.sync.dma_start(xh16[:, :, :], T["xh_d"].ap().rearrange("c p q -> p c q"))
        nc.sync.dma_start(wvm[:, :, :], T["wvm_d"].ap().rearrange("c p q -> p c q"))

        nc.sync.dma_start(wfT[:, :], T["wfT_d"].ap())
        nc.sync.dma_start(iot[:, :], T["iota_d"].ap())
        nc.sync.dma_start(qb8[:, :], T["qb8_d"].ap())
        nc.sync.dma_start(fr0[:, :, :], T["fpad_d"].ap()[0])
        nc.sync.dma_start(fr1[:, :, :], T["fpad_d"].ap()[1])
        nc.vector.memset(ct2[:, :, :], 0.0)
        nc.vector.memset(ct3[:, :, :], 0.0)
        # vdram zero row
        vz = S.tile([1, C], F16, tag="vz")
        nc.vector.memset(vz[:, :], 0.0)
        nc.sync.dma_start(vdram[HWF:VROWS, :], vz[:, :])

        # ---------- qk (fp16 hi/lo x hi/lo) + vT ----------
        with tc.tile_pool(name="psqk", bufs=2, space="PSUM") as PSQK, \
             tc.tile_pool(name="psv", bufs=2, space="PSUM") as PSV, \
             tc.tile_pool(name="vstg", bufs=2) as VS:

            def qk_mm(which, xh_t, xl_t, npos, stack, hrow, lrow):
                nchunks = (npos + 511) // 512
                for ch in range(nchunks):
                    n0, n1 = ch * 512, min((ch + 1) * 512, npos)
                    pq = PSQK.tile([C8, 512], F32, tag="psqk")
                    first = True
                    for cb in range(2):
                        for wsplit in range(2):
                            wcol = ((which * 2 + cb) * 2 + wsplit) * C8
                            for xs, xt in ((0, xh_t), (1, xl_t)):
                                nc.tensor.matmul(
                                    pq[:, 0:n1 - n0],
                                    wqkT[:, wcol:wcol + C8],
                                    xt[:, cb, n0:n1],
                                    start=first,
                                    stop=(cb == 1 and wsplit == 1 and xs == 1))
                                first = False
                    if has_bqk:
                        nc.vector.tensor_scalar(
                            out=stack[hrow:hrow + C8, n0:n1], in0=pq[:, 0:n1 - n0],
                            scalar1=bqs[:, which:which + 1], scalar2=None, op0=ALU.add)
                    else:
                        nc.scalar.copy(stack[hrow:hrow + C8, n0:n1], pq[:, 0:n1 - n0])
                    nc.vector.scalar_tensor_tensor(
                        stack[lrow:lrow + C8, n0:n1], pq[:, 0:n1 - n0],
                        bqs[:, which:which + 1] if has_bqk else 0.0,
                        stack[hrow:hrow + C8, n0:n1],
                        op0=ALU.add, op1=ALU.subtract)

            # qstack rows: [qh,ql] then DMA-replicate; kstack rows: [kh,_,kl,_]
            qk_mm(0, cxh, cxl, EXTQ, qstack, hrow=0, lrow=32)
            qk_mm(1, fxh, fxl, HWF, kstack, hrow=0, lrow=64)
            nc.sync.dma_start(qstack[64:128, :], qstack[0:64, :])
            nc.sync.dma_start(kstack[32:64, :], kstack[0:32, :])
            nc.sync.dma_start(kstack[96:128, :], kstack[64:96, :])

            # vT: psum [128 keys, 2*C] per key-block pair -> vstage -> vdram
            for kb8 in range(4):
                vstage = VS.tile([128, 8, C], F16, tag="vstg")
                for kbl in range(8):
                    kb = kb8 * 8 + kbl
                    if kbl % 2 == 0:
                        pv = PSV.tile([128, 512], F32, tag="psv")
                    half = (kbl % 2) * C
                    for cb in range(2):
                        nc.tensor.matmul(
                            pv[:, half:half + C],
                            xh16[:, cb, kb * 128:(kb + 1) * 128],
                            wvm[:, cb, :],
                            start=(cb == 0), stop=(cb == 1))
                    if kbl % 2 == 1:
                        nc.scalar.copy(
                            vstage[:, kbl - 1:kbl + 1, :].rearrange("p a c -> p (a c)"),
                            pv[:, :])
                nc.sync.dma_start(
                    vdram[kb8 * 1024:(kb8 + 1) * 1024, :]
                    .rearrange("(kbl p) c -> p kbl c", p=128),
                    vstage[:, :, :])

        XHP_cm.__exit__(None, None, None)
        QKP_cm.__exit__(None, None, None)

        # ---------- energy + hierarchical argmax + pipelined gathers ----------
        front_pairs = [(cb4, tap) for cb4 in (0, 1) for tap in range(9)]
        t_pairs = [(cb4, tap) for cb4 in (2, 3) for tap in range(9)]
        cats = [fr0, fr1, ct2, ct3]
        NBATCH = (NBLK + 3) // 4          # 5 (4,4,4,4,1 blocks)

        wcS = SM[:, 91:91 + NBLK]
        lidxS = SM[:, 108:108 + NBLK]

        with tc.tile_pool(name="pcv0", bufs=4, space="PSUM") as PCV0:
            pcs0 = [PCV0.tile([128, 512], F32, tag="pcv0", name=f"pcv0_{g}")
                    for g in range(4)]

            def conv_front_pair(pcs, pair):
                cb4, tap = pair
                dy, dx = tap // 3, tap % 3
                col = ((cb4 * 9 + tap) * 2 + 0) * 128
                for g in range(4):
                    nc.tensor.matmul(
                        pcs[g][:, :], wfT[:, col:col + 128],
                        cats[cb4][:, g * 8 + dy:g * 8 + dy + 8, dx:dx + W],
                        start=(pair == front_pairs[0]), stop=False)

            def conv_g(pcs, ob, g, pairs, start_pair, stop_pair):
                for (cb4, tap) in pairs:
                    dy, dx = tap // 3, tap % 3
                    col = ((cb4 * 9 + tap) * 2 + ob) * 128
                    nc.tensor.matmul(
                        pcs[g][:, :], wfT[:, col:col + 128],
                        cats[cb4][:, g * 8 + dy:g * 8 + dy + 8, dx:dx + W],
                        start=((cb4, tap) == start_pair),
                        stop=((cb4, tap) == stop_pair))

            def out_stage_g(pcs, ob, g):
                fcat = cats[ob]
                stage = S.tile([128, 512], F32, tag="ostage", name=f"ost{ob}{g}")
                nc.vector.scalar_tensor_tensor(
                    stage[:, :], pcs[g][:, :], bfs[:, ob:ob + 1],
                    s128[:, W + g * 512:W + (g + 1) * 512],
                    op0=ALU.add, op1=ALU.mult)
                nc.vector.tensor_tensor(
                    stage[:, :], stage[:, :],
                    fcat[:, g * 8 + 1:g * 8 + 9, 1:1 + W], op=ALU.add)
                nc.sync.dma_start(
                    T["out_d"].ap()[ob][:, g * 512:(g + 1) * 512], stage[:, :])

            # all 18 front pairs must land in blocks 0..11 (conv-T g0 stops
            # the pcv0 groups at b==12)
            fp_sched = [0, 2, 4, 6, 8, 10, 12, 13, 14, 15, 16, 17, 18] + [18] * 5

            wts = [None] * NBATCH

            def batch_issue(i):
                # winner flat idx (q*8 + wc) -> wrapped -> window gather
                c0 = 4 * i
                c1 = min(c0 + 4, NBLK)
                nb = c1 - c0
                s0, s1 = c0 * 8, c1 * 8
                w0 = c0 * 128
                idxf = S.tile([128, 4], F32, tag="idxf", name=f"idxf{i}")
                nc.vector.tensor_tensor(idxf[:, 0:nb], wcS[:, c0:c1],
                                        qb8[:, c0:c1], op=ALU.add)
                iwc = S.tile([128, 4], I16, tag="iwc", name=f"iwc{i}")
                nc.vector.tensor_copy(iwc[:, 0:nb], idxf[:, 0:nb])
                nc.sync.dma_start(
                    wrapW[w0:w0 + nb * 128].rearrange("(b p) -> p b", p=128),
                    iwc[:, 0:nb])
                nc.sync.dma_start(
                    widxP[0:16, s0:s1],
                    wrapW[w0:w0 + nb * 128].rearrange("(s p0) -> p0 s", p0=16))
                nc.sync.dma_start(widxP[16:32, s0:s1], widxP[0:16, s0:s1])
                nc.sync.dma_start(widxP[32:64, s0:s1], widxP[0:32, s0:s1])
                nc.sync.dma_start(widxP[64:128, s0:s1], widxP[0:64, s0:s1])
                wt = WT.tile([128, nb, 512], F32, tag=f"wt{nb}", name=f"wt{i}")
                nc.gpsimd.dma_gather(
                    wt[:, :, :], edram[i * 4096:i * 4096 + nb * 1024, :],
                    widxP[:, s0:s1],
                    nb * 128, nb * 128, elem_size=512, transpose=False)
                wts[i] = wt

            def batch_windows(i):
                c0 = 4 * i
                c1 = min(c0 + 4, NBLK)
                nb = c1 - c0
                s0, s1 = c0 * 8, c1 * 8
                w0 = c0 * 128
                for j, b in enumerate(range(c0, c1)):
                    m512 = MKS.tile([128, 512], F16, tag="m512")
                    nc.vector.tensor_scalar(
                        out=m512[:, :], in0=wts[i][:, j, :],
                        scalar1=Mg[:, b:b + 1], scalar2=None, op0=ALU.is_equal)
                    scr = MKS.tile([128, 512], F16, tag="scr512")
                    nc.vector.scalar_tensor_tensor(
                        scr[:, :], m512[:, :], 1.0, iot[:, 0:512],
                        op0=ALU.mult, op1=ALU.mult,
                        accum_out=lidxS[:, b:b + 1])
                # Ag = wc*512 + lidx; mask halo queries to the zero row
                nc.vector.scalar_tensor_tensor(
                    Agf[:, c0:c1], wcS[:, c0:c1], 512.0, lidxS[:, c0:c1],
                    op0=ALU.mult, op1=ALU.add)
                nc.vector.tensor_tensor(Agm[:, c0:c1], Agf[:, c0:c1],
                                        maskt[:, c0:c1], op=ALU.mult)
                nc.vector.tensor_tensor(Agm[:, c0:c1], Agm[:, c0:c1],
                                        amaskt[:, c0:c1], op=ALU.add)
                nc.vector.tensor_scalar(
                    out=Agm[:, c0:c1], in0=Agm[:, c0:c1], scalar1=float(HWF),
                    scalar2=None, op0=ALU.min)
                itT = S.tile([128, 4], I16, tag="itT", name=f"itT{i}")
                nc.vector.tensor_copy(itT[:, 0:nb], Agm[:, c0:c1])
                nc.sync.dma_start(
                    wrap_t[w0:w0 + nb * 128].rearrange("(b p) -> p b", p=128),
                    itT[:, 0:nb])
                nc.sync.dma_start(
                    idxw[0:16, s0:s1],
                    wrap_t[w0:w0 + nb * 128].rearrange("(s p0) -> p0 s", p0=16))
                nc.sync.dma_start(idxw[16:32, s0:s1], idxw[0:16, s0:s1])
                nc.sync.dma_start(idxw[32:64, s0:s1], idxw[0:32, s0:s1])
                nc.sync.dma_start(idxw[64:128, s0:s1], idxw[0:64, s0:s1])
                if os.environ.get("KV_NO_GATHER"):
                    nc.vector.memset(tgp[i][:, :, :], 0.0)
                else:
                    nc.gpsimd.dma_gather(
                        tgp[i][:, :, :], vdram[:, :], idxw[:, s0:s1],
                        nb * 128, nb * 128, elem_size=C, transpose=True)
                if has_bv:
                    for cb in range(2):
                        nc.vector.tensor_scalar(
                            out=tgp[i][:, cb, :], in0=tgp[i][:, cb, :],
                            scalar1=bvs[:, cb:cb + 1], scalar2=None, op0=ALU.add)
                for cb, ct in ((0, ct2), (1, ct3)):
                    nc.vector.tensor_copy(
                        ct[:, 2 * c0:2 * c1, 1:W + 1],
                        tgp[i][:, cb, :].rearrange("p (r w) -> p r w", w=W))

            with tc.tile_pool(name="pse", bufs=2, space="PSUM") as PSE, \
                 tc.tile_pool(name="ebufp", bufs=2) as EB, \
                 tc.tile_pool(name="cmp", bufs=2) as CM, \
                 tc.tile_pool(name="mks", bufs=2) as MKS, \
                 tc.tile_pool(name="wtp", bufs=2) as WT:
                for b in range(NBLK):
                    ebuf = EB.tile([128, HWF], F32, tag="ebuf")
                    for c in range(4):
                        pe = PSE.tile([128, 1024], F32, tag="pse")
                        nc.tensor.matmul(pe[:, 0:512],
                                         qstack[:, b * 128:(b + 1) * 128],
                                         kstack[:, c * 1024:c * 1024 + 512],
                                         start=True, stop=True)
                        nc.tensor.matmul(pe[:, 512:1024],
                                         qstack[:, b * 128:(b + 1) * 128],
                                         kstack[:, c * 1024 + 512:(c + 1) * 1024],
                                         start=True, stop=True)
                        nc.scalar.copy(ebuf[:, c * 1024:(c + 1) * 1024], pe[:, :])
                        if c == 1:
                            for pi in range(fp_sched[b], fp_sched[b + 1]):
                                conv_front_pair(pcs0, front_pairs[pi])
                    nc.scalar.dma_start(
                        edram[b * 1024:(b + 1) * 1024, :]
                        .rearrange("(p c) k -> p (c k)", p=128),
                        ebuf[:, :])
                    cm8 = CM.tile([128, 8], F32, tag="cm8")
                    nc.vector.tensor_reduce(
                        cm8[:, :], ebuf[:, :].rearrange("p (c k) -> p c k", c=8),
                        axis=AX.X, op=ALU.max)
                    nc.vector.tensor_reduce(Mg[:, b:b + 1], cm8[:, :],
                                            axis=AX.X, op=ALU.max)
                    m8 = CM.tile([128, 8], F16, tag="m8")
                    nc.vector.tensor_scalar(
                        out=m8[:, :], in0=cm8[:, :], scalar1=Mg[:, b:b + 1],
                        scalar2=None, op0=ALU.is_equal)
                    s8 = CM.tile([128, 8], F16, tag="s8")
                    nc.vector.scalar_tensor_tensor(
                        s8[:, :], m8[:, :], 1.0, iot[:, 0:8],
                        op0=ALU.mult, op1=ALU.mult, accum_out=wcS[:, b:b + 1])
                    nc.vector.tensor_scalar(
                        out=wcS[:, b:b + 1], in0=wcS[:, b:b + 1], scalar1=7.0,
                        scalar2=None, op0=ALU.min)
                    if b % 4 == 3 or b == NBLK - 1:
                        batch_issue(b // 4)
                    if b >= 5 and (b - 5) % 4 == 0:
                        batch_windows((b - 5) // 4)
                    if b == 12:
                        conv_g(pcs0, 0, 0, t_pairs, None, t_pairs[-1])
                    if b == 14:
                        conv_g(pcs0, 0, 1, t_pairs, None, t_pairs[-1])
                # tail: remaining windows, dbg, s128, conv-T g2/g3 + ob0 out
                batch_windows(3)
                batch_windows(4)
                nc.sync.dma_start(T["dbg_s_d"].ap(), Mg[:, :])
                nc.sync.dma_start(T["dbg_arg_d"].ap(), Agf[:, :])

                ptx = PSE.tile([128, 1024], F32, tag="pse")
                nc.tensor.transpose(ptx[0:NBLK, 0:128], Mg[:, :], ident[:, :])
                srow_stage = S.tile([NBLK, 128], F32, tag="srowstg")
                nc.scalar.copy(srow_stage[:, :], ptx[0:NBLK, 0:128])
                nc.sync.dma_start(
                    srow_t[:].rearrange("(b p) -> b p", p=128), srow_stage[:, :])
                nc.sync.dma_start(
                    s128[:, :],
                    srow_t[:].rearrange("q -> () q").broadcast_to((128, EXTQ)))

                conv_g(pcs0, 0, 2, t_pairs, None, t_pairs[-1])
                conv_g(pcs0, 0, 3, t_pairs, None, t_pairs[-1])

            # ---------- conv ob1 + out stages ----------
            all_pairs = [(cb4, tap) for cb4 in range(4) for tap in range(9)]
            with tc.tile_pool(name="pcv1", bufs=4, space="PSUM") as PCV1:
                pcs1 = [PCV1.tile([128, 512], F32, tag="pcv1", name=f"pcv1_{g}")
                        for g in range(4)]
                for g in range(4):
                    out_stage_g(pcs0, 0, g)
                    conv_g(pcs1, 1, g, all_pairs, all_pairs[0], all_pairs[-1])
                for g in range(4):
                    out_stage_g(pcs1, 1, g)


def _f16_split(x):
    h = x.astype(np.float16)
    l = (x - h.astype(np.float32)).astype(np.float16)
    return h, l


def _prep_core_inputs(inputs, core):
    b, half = core // 2, core % 2
    r0 = half * RH

    def ext_rows(x, wpad=False):  # (C,H,W) -> (C,EXTR,W[+2]) zero boundary
        w = WP if wpad else W
        out = np.zeros((C, EXTR, w), np.float32)
        lo, hi = r0 - 1, r0 + RH + 1
        slo, dlo = max(lo, 0), max(lo, 0) - lo
        shi = min(hi, H)
        if wpad:
            out[:, dlo:dlo + shi - slo, 1:W + 1] = x[:, slo:shi]
        else:
            out[:, dlo:dlo + shi - slo] = x[:, slo:shi]
        return out

    cxe = ext_rows(inputs["cross_x"][b]).reshape(2, 128, EXTQ)
    cxh, cxl = _f16_split(cxe)
    fx = inputs["front_x"][b].reshape(2, 128, HWF)
    fxh, fxl = _f16_split(fx)
    xh16 = inputs["front_x_hat"][b].reshape(2, 128, HWF).astype(np.float16)
    fpad16 = ext_rows(inputs["front_x"][b], wpad=True).reshape(
        2, 128, EXTR, WP).astype(np.float16)

    # wqkT: [128, ((which*2+cb)*2+hl)*32 + o]
    wqkT = np.zeros((128, 8 * C8), np.float16)
    for which, wname in ((0, "Wq"), (1, "Wk")):
        wh, wl = _f16_split(inputs[wname])          # (32, 256)
        for cb in range(2):
            for hl, warr in ((0, wh), (1, wl)):
                col = ((which * 2 + cb) * 2 + hl) * C8
                wqkT[:, col:col + C8] = warr[:, cb * 128:(cb + 1) * 128].T

    # wvm[cb][ci, oc] = Wv[oc, cb*128+ci]
    wv = inputs["Wv"]                                # (256, 256)
    wvm = np.stack([wv[:, 0:128].T, wv[:, 128:256].T]).astype(np.float16)

    # wfT[ci, ((cb4*9+tap)*2+ob)*128 + oc]
    wf = inputs["Wf"].reshape(C, 2 * C, 9)           # (oc, ci, tap)
    arr = wf.transpose(1, 2, 0)                      # (ci, tap, oc)
    arr = arr.reshape(4, 128, 9, 2, 128)             # (cb4, ci, tap, ob, oc)
    wfT = np.ascontiguousarray(
        arr.transpose(1, 0, 2, 3, 4).reshape(128, 72 * 128)).astype(np.float16)

    iota = np.broadcast_to(np.arange(HWF, dtype=np.int16), (128, HWF))
    qb8 = ((np.arange(NBLK)[None, :] % 4) * 128
           + np.arange(128)[:, None]).astype(np.float32) * 8.0

    valid = np.ones((EXTR, W), np.float32)
    if r0 == 0:
        valid[0] = 0.0
    if r0 + RH == H:
        valid[-1] = 0.0
    vq = valid.reshape(EXTQ)
    mask = np.empty((128, NBLK), np.float32)
    for blk in range(NBLK):
        mask[:, blk] = vq[blk * 128:(blk + 1) * 128]
    amask = (1.0 - mask) * HWF

    return {
        "cxh": cxh, "cxl": cxl, "fxh": fxh, "fxl": fxl, "xh": xh16,
        "fpad": fpad16, "wqkT": wqkT, "wvm": wvm, "wfT": wfT,
        "iota": np.ascontiguousarray(iota), "qb8": qb8,
        "mask": mask, "amask": amask,
        "bq": inputs["bq"].reshape(C8, 1), "bk": inputs["bk"].reshape(C8, 1),
        "bv": np.ascontiguousarray(inputs["bv"].reshape(2, 128).T),
        "bf": np.ascontiguousarray(inputs["bf"].reshape(2, 128).T),
        "ident": np.eye(128, dtype=np.float32),
    }


LAST_RES = None


def kernel(_trace=False, **inputs):
    global LAST_RES
    inputs = {k: np.asarray(v, dtype=np.float32) for k, v in inputs.items()}
    has_bqk = bool(np.any(inputs["bq"]) or np.any(inputs["bk"]))
    has_bv = bool(np.any(inputs["bv"]))
    nc = _build(has_bqk, has_bv)
    in_maps = [_prep_core_inputs(inputs, core) for core in range(8)]
    kw = {"trace": True} if _trace else {}
    res = bass_utils.run_bass_kernel_spmd(nc, in_maps, core_ids=list(range(8)), **kw)
    LAST_RES = res
    out = np.empty((B, C, H, W), np.float32)
    for core in range(8):
        b, half = core // 2, core % 2
        o = res.results[core]["out"].reshape(C, RH, W)
        out[b, :, half * RH:(half + 1) * RH, :] = o
    return out


if __name__ == "__main__":
    rng = np.random.default_rng(0)
    ins = {
        "front_x": rng.standard_normal((B, C, H, W)).astype(np.float32),
        "cross_x": rng.standard_normal((B, C, H, W)).astype(np.float32),
        "front_x_hat": rng.standard_normal((B, C, H, W)).astype(np.float32),
        "Wq": (rng.standard_normal((C8, C)) / 16).astype(np.float32),
        "bq": np.zeros((C8,), np.float32),
        "Wk": (rng.standard_normal((C8, C)) / 16).astype(np.float32),
        "bk": np.zeros((C8,), np.float32),
        "Wv": (rng.standard_normal((C, C)) / 16).astype(np.float32),
        "bv": np.zeros((C,), np.float32),
        "Wf": (rng.standard_normal((C, 2 * C, 3, 3)) / 68).astype(np.float32),
        "bf": np.zeros((C,), np.float32),
    }
    out = kernel(**ins)
    print("kernel ran, out shape", out.shape, "std", out.std())
